# revision 19
# baseline (speedup 1.0000x reference)
"""Bilateral filter (nn_BilateralFilter) Trainium2 Bass kernel.

Reference semantics (KERNEL_SIZE=5, THETA_ALPHA=2.0, THETA_BETA=0.1):
    w_k   = exp(-(dx^2+dy^2)/8)                      (24 offsets, center dropped)
    Ki    = exp(-50*(I(p+k) - I(p))^2)               per image channel c
    out[c,n,p] = sum_k w_k*Ki[c,k,p]*Q(n,p+k) / sum_k w_k*Ki[c,k,p]

Sharding: 8 cores = 2 batches x 4 row-slabs of 80 output rows.  On-chip the
slab is processed as 2 half-slabs with partitions = (3 channels x 40 rows)
= 120 of 128 lanes (Q is replicated 3x across the channel blocks).

Engine split (vs the all-DVE baseline):
  DVE : neighbor subs + the 24x(c,n)-product planes (fp16, 2x mode) + recip
        and the final psum*rnorm scale
  ACT : Square + Exp with the spatial weight folded into per-group exp bias
        (slots grouped by (dr^2+dc^2) so one strided activation covers a
        +/-dc pair); also upconverts the uint8 Q blocks to fp16 and applies
        the 1/255 scale to the reciprocal
  PE  : every k-fold (numerator and norm) as identity-weight matmuls
        accumulating into PSUM fp32 (6 numerator banks + 1 norm bank)
  DMA : Q ships as uint8 (4x less HBM/queue traffic; quantization ~4e-3
        relative, folded out via rnorm/255); blocks are split into column
        slices round-robin over the gpsimd/sync queues

The center slot is excluded from both folds (reference drops it); its kw
value stays exactly 0 because d=0 -> Square -> 0 and Exp is never applied.
The +5 exponent shift keeps kw*Q255 inside fp16 range and cancels in the
final division.  The output also ships as uint8 (the numerator's x255
cancels against it; a 255/256 guard prevents wrap), quartering the final
store's DMA tail.

Measured: ~90 us on HW (max core), rel err ~2.4e-3 vs the fp32 reference
(budget 2e-2; dominated by the uint8 Q/out quantization, each ~2e-3).
"""

import numpy as np

B, C, NCL = 2, 3, 6
H = W = 320
KS, PAD = 5, 2
WP = W + 2 * PAD          # 324
NSLAB = 4
R = H // NSLAB            # 80 output rows per shard
RH = R + 2 * PAD          # 84 rows incl. halo
HALF = R // 2             # 40 rows per half-slab
NP = C * HALF             # 120 partitions
COEF = 50.0               # 1/(2*theta_beta^2)
SHIFT = 5.0               # exponent shift, cancels in the division
IWH = KS * WP             # Ia free width per half: 5 dr blocks  (1620)
QW1 = NCL * WP            # one Q dr block row: 6 n x 324        (1944)
DW = KS * W               # d/sq free width per dr block = 5*320 (1600)
SW = KS * KS * W          # sq tile free width, 25 slots         (8000)
PW = KS * NCL * W         # P5 free width: 5 dc x 6 n x 320      (9600)
NW = NCL * W              # 1920

_CACHE: dict = {}

# dr processing order: center block first (every sub reads it)
DRS = [2, 0, 1, 3, 4]


def _emit(tc, i_ap, q_ap, e_ap, out_ap):
    """Emit the per-core program into TileContext tc.

    i_ap:   DRAM AP (2*KS*NP*WP,)   fp16   image blocks, zero-padded
    q_ap:   DRAM AP (2*KS*NP*QW1,)  uint8  Q*255 blocks, zero-padded
    e_ap:   DRAM AP (NP*NP,)        fp16   identity matrix
    out_ap: DRAM AP (R*C*NCL*W,)    fp16   output rows x (c, n, x)
    """
    import concourse.bass as bass
    import concourse.mybir as mybir

    f16 = mybir.dt.float16
    f32 = mybir.dt.float32
    u8 = mybir.dt.uint8
    AF = mybir.ActivationFunctionType
    nc = tc.nc

    def fold_slots():
        return [
            (dr, dc)
            for dr in DRS
            for dc in range(KS)
            if not (dr == PAD and dc == PAD)
        ]

    with tc.tile_pool(name="p", bufs=1) as pool, tc.tile_pool(
        name="pp", bufs=1, space="PSUM"
    ) as ppool:
        # exp biases: b = SHIFT - s/8 for s = (dr-2)^2 + (dc-2)^2
        svals = [1, 2, 4, 5, 8]
        bias_t = pool.tile([NP, len(svals)], f32, tag="bias")
        bcol = {}
        for j, s in enumerate(svals):
            bcol[s] = j
            nc.gpsimd.memset(bias_t[:, j : j + 1], SHIFT - s / 8.0)

        ident = pool.tile([NP, NP], f16, tag="ident")
        nc.sync.dma_start(ident[:, :], e_ap)

        Ia = [pool.tile([NP, IWH], f16, tag=f"Ia{h}", name=f"Ia{h}") for h in range(2)]
        Qu = [pool.tile([NP, KS * QW1], u8, tag=f"Qu{h}", name=f"Qu{h}") for h in range(2)]

        # split big blocks into column slices, round-robin across the two
        # idle DMA queues (gpsimd + sync); never trigger from the ACT/DVE
        # queues, whose sequencers carry critical-path compute
        queues = [nc.gpsimd, nc.sync]
        qi = [0]

        def dma(dst, src):
            queues[qi[0] % len(queues)].dma_start(dst, src)
            qi[0] += 1

        def load_q(h, dr, nsplit):
            base = (h * KS + dr) * NP * QW1
            step = QW1 // nsplit
            for j in range(nsplit):
                dma(
                    Qu[h][:, dr * QW1 + j * step : dr * QW1 + (j + 1) * step],
                    bass.AP(
                        tensor=q_ap.tensor,
                        offset=q_ap.offset + base + j * step,
                        ap=[[QW1, NP], [1, step]],
                    ),
                )

        def load_i(h, dr, nsplit=1):
            base = (h * KS + dr) * NP * WP
            step = WP // nsplit
            rem = WP - step * (nsplit - 1)
            for j in range(nsplit):
                w = step if j < nsplit - 1 else rem
                dma(
                    Ia[h][:, dr * WP + j * step : dr * WP + j * step + w],
                    bass.AP(
                        tensor=i_ap.tensor,
                        offset=i_ap.offset + base + j * step,
                        ap=[[WP, NP], [1, w]],
                    ),
                )

        # first-needed first: the dr=2 blocks gate the whole front
        load_i(0, PAD, 2)
        load_q(0, PAD, 4)
        for dr in DRS:
            if dr != PAD:
                load_i(0, dr)
        for dr in DRS:
            if dr != PAD:
                load_q(0, dr, 2)
        for dr in DRS:
            load_i(1, dr)
        load_q(1, PAD, 2)
        for dr in DRS:
            if dr != PAD:
                load_q(1, dr, 2)

        sq = [pool.tile([NP, SW], f16, tag=f"sq{h}", name=f"sq{h}") for h in range(2)]
        d_t = pool.tile([NP, SW], f16, tag="d")  # shared scratch across halves

        def emit_front(h):
            """subs (DVE) + square/exp + Q upconvert (ACT) for half h."""
            ia, sqh = Ia[h], sq[h]
            # d[dr block] = I(p + (dr, dc)) - I(p).  The dr=2 sub goes first
            # and alone (it gates the whole ACT front); the other four drs
            # merge into one 4-dim-AP op (dr in the free dims) to cut DVE
            # dispatch and semaphore overhead.
            def sub_op(dr0, ndr, drstep):
                dst = bass.AP(
                    tensor=d_t.tensor,
                    offset=d_t.offset + dr0 * DW,
                    ap=[[SW, NP], [drstep * DW, ndr], [W, KS], [1, W]],
                )
                in0 = bass.AP(
                    tensor=ia.tensor,
                    offset=ia.offset + dr0 * WP,
                    ap=[[IWH, NP], [drstep * WP, ndr], [1, KS], [1, W]],
                )
                in1 = bass.AP(
                    tensor=ia.tensor,
                    offset=ia.offset + PAD * WP + PAD,
                    ap=[[IWH, NP], [0, ndr], [0, KS], [1, W]],
                )
                nc.vector.tensor_sub(dst, in0, in1)

            sub_op(PAD, 1, 1)      # dr=2
            sub_op(0, 2, 1)        # dr=0,1
            sub_op(3, 2, 1)        # dr=3,4
            qf = {}
            for dr in DRS:
                # upconvert this dr's Q block u8 -> f16 (ACT) first: it only
                # needs the DMA, so it never delays the exp that gates the
                # product, and the product needs both
                qb = pool.tile([NP, QW1], f16, tag="Qf", bufs=3)
                nc.scalar.copy(qb[:, :], Qu[h][:, dr * QW1 : (dr + 1) * QW1])
                qf[dr] = qb
                nc.scalar.activation(
                    sqh[:, dr * DW : (dr + 1) * DW],
                    d_t[:, dr * DW : (dr + 1) * DW],
                    AF.Square,
                )
                # exp in place, slots grouped by |dc-2|: {0,4}, {1,3}, {2}
                for dcs, ds2 in (((0, 4), 4), ((1, 3), 1), ((2,), 0)):
                    if dr == PAD and ds2 == 0:
                        continue  # center slot stays 0
                    s = (dr - PAD) ** 2 + ds2
                    j = bcol[s]
                    if len(dcs) == 2:
                        ap_dims = [[SW, NP], [(dcs[1] - dcs[0]) * W, 2], [1, W]]
                    else:
                        ap_dims = [[SW, NP], [1, W]]
                    src = bass.AP(
                        tensor=sqh.tensor,
                        offset=sqh.offset + dr * DW + dcs[0] * W,
                        ap=ap_dims,
                    )
                    nc.scalar.activation(
                        src, src, AF.Exp, bias=bias_t[:, j : j + 1], scale=-COEF
                    )
            return qf

        def emit_products(h, qf, psum_n, psum_nrm):
            """products (DVE) + all fold matmuls (PE)."""
            kw = sq[h]
            slots = fold_slots()
            first, last = slots[0], slots[-1]
            for dr in DRS:
                qb = qf[dr]
                p5 = pool.tile([NP, PW], f16, tag="P5", bufs=4)
                out = bass.AP(
                    tensor=p5.tensor,
                    offset=p5.offset,
                    ap=[[PW, NP], [NW, KS], [W, NCL], [1, W]],
                )
                in0 = bass.AP(
                    tensor=kw.tensor,
                    offset=kw.offset + dr * DW,
                    ap=[[SW, NP], [W, KS], [0, NCL], [1, W]],
                )
                in1 = bass.AP(
                    tensor=qb.tensor,
                    offset=qb.offset,
                    ap=[[QW1, NP], [1, KS], [WP, NCL], [1, W]],
                )
                nc.vector.tensor_mul(out, in0, in1)
                for dc in range(KS):
                    if dr == PAD and dc == PAD:
                        continue
                    st = (dr, dc) == first
                    sp = (dr, dc) == last

                    def nrm_mm():
                        nc.tensor.matmul(
                            psum_nrm[:, :],
                            ident[:, :],
                            kw[:, (dr * KS + dc) * W : (dr * KS + dc + 1) * W],
                            start=st,
                            stop=sp,
                        )

                    # in the closing slot, stop the norm chain first so the
                    # reciprocal unblocks before the finals' banks
                    if sp:
                        nrm_mm()
                    for n in range(NCL):
                        nc.tensor.matmul(
                            psum_n[n][:, :],
                            ident[:, :],
                            p5[:, dc * NW + n * W : dc * NW + (n + 1) * W],
                            start=st,
                            stop=sp,
                        )
                    if not sp:
                        nrm_mm()

        def emit_readout(h, psum_n, psum_nrm):
            """recip (DVE), guard scale (ACT), final scale (DVE), store.

            The output ships as uint8: the numerator already carries the
            x255 from the Q quantization, so psum*rnorm is in [0, 255];
            the 255/256 guard keeps it under 255 so the u8 convert can
            never wrap, and the host folds 256/255^2 back out."""
            rnorm = pool.tile([NP, W], f32, tag=f"rn{h}")
            nc.vector.reciprocal_approx_fast(rnorm[:, :], psum_nrm[:, :])
            rns = pool.tile([NP, W], f32, tag=f"rns{h}")
            nc.scalar.mul(rns[:, :], rnorm[:, :], 255.0 / 256.0)
            ot = pool.tile([NP, NW], u8, tag=f"ot{h}")
            for n in range(NCL):
                nc.vector.tensor_mul(
                    ot[:, n * W : (n + 1) * W], psum_n[n][:, :], rns[:, :]
                )
                dma(
                    bass.AP(
                        tensor=out_ap.tensor,
                        offset=out_ap.offset + h * HALF * C * NW + n * W,
                        ap=[[NW, C], [C * NW, HALF], [1, W]],
                    ),
                    ot[:, n * W : (n + 1) * W],
                )

        # PSUM tiles shared across halves (bufs=1): half B's first matmul on
        # a bank waits for half A's readout of that bank -> per-bank pipelining
        psum_n = [ppool.tile([NP, W], f32, tag=f"ps{n}", name=f"ps{n}") for n in range(NCL)]
        psum_nrm = ppool.tile([NP, W], f32, tag="psn")

        qf0 = emit_front(0)
        emit_products(0, qf0, psum_n, psum_nrm)
        qf1 = emit_front(1)
        with tc.high_priority():
            emit_readout(0, psum_n, psum_nrm)
        emit_products(1, qf1, psum_n, psum_nrm)
        emit_readout(1, psum_n, psum_nrm)


def _build_program():
    import concourse.bacc as bacc
    import concourse.mybir as mybir
    from concourse import tile

    f16 = mybir.dt.float16
    u8 = mybir.dt.uint8

    nc = bacc.Bacc("TRN2", num_devices=8, debug=False)
    I_in = nc.dram_tensor("i_in", [2 * KS * NP * WP], f16, kind="ExternalInput")
    Q_in = nc.dram_tensor("q_in", [2 * KS * NP * QW1], u8, kind="ExternalInput")
    E_in = nc.dram_tensor("ident", [NP, NP], f16, kind="ExternalInput")
    OUT = nc.dram_tensor("out", [R * C * NCL * W], u8, kind="ExternalOutput")

    with tile.TileContext(nc) as tc:
        _emit(tc, I_in.ap(), Q_in.ap(), E_in.ap(), OUT.ap())

    nc.compile()
    return nc


def _get_program():
    if "nc" not in _CACHE:
        _CACHE["nc"] = _build_program()
    return _CACHE["nc"]


def _shard_inputs(Q, I):
    """Host prep: pad, quantize Q to uint8 (Q*255), cast I to fp16, and
    pre-lay each core's inputs in the exact on-chip layout
    [half, dr, (c,rr) partitions, free] so device DMAs are contiguous.
    Q rows are replicated across the 3 channel blocks."""
    Qp = np.pad(
        np.asarray(Q, np.float32), ((0, 0), (0, 0), (PAD, PAD), (PAD, PAD))
    )
    Qu8 = np.rint(Qp * 255.0).astype(np.uint8)
    Ip = np.pad(
        np.asarray(I, np.float32), ((0, 0), (0, 0), (PAD, PAD), (PAD, PAD))
    ).astype(np.float16)
    eye = np.eye(NP, dtype=np.float16)
    in_maps = []
    for b in range(B):
        for s in range(NSLAB):
            r0 = s * R
            i_blk = np.empty((2, KS, NP, WP), np.float16)
            q_blk = np.empty((2, KS, NP, QW1), np.uint8)
            for h in range(2):
                for dr in range(KS):
                    rr = r0 + h * HALF + dr
                    i_blk[h, dr] = Ip[b, :, rr : rr + HALF, :].reshape(NP, WP)
                    qb = Qu8[b, :, rr : rr + HALF, :].transpose(1, 0, 2)
                    q_blk[h, dr] = np.tile(
                        qb.reshape(1, HALF, QW1), (C, 1, 1)
                    ).reshape(NP, QW1)
            in_maps.append(
                {
                    "i_in": i_blk.reshape(-1),
                    "q_in": q_blk.reshape(-1),
                    "ident": eye,
                }
            )
    return in_maps


def _assemble(outs):
    # outs: list of 8 uint8 arrays (R*C*NCL*W,), core order = (b, slab)
    o = np.stack([np.asarray(x) for x in outs]).astype(np.float32)
    o *= 256.0 / (255.0 * 255.0)
    o = o.reshape(B, NSLAB, R, C, NCL, W)
    o = o.transpose(0, 3, 4, 1, 2, 5).reshape(B, C, NCL, H, W)
    return o


def run(Q, I, trace=False):
    from concourse.bass_utils import run_bass_kernel_spmd

    nc = _get_program()
    in_maps = _shard_inputs(Q, I)
    res = run_bass_kernel_spmd(nc, in_maps, list(range(8)), trace=trace)
    out = _assemble([res.results[i]["out"] for i in range(8)])
    return out, res


def kernel(Q, I):
    out, _ = run(Q, I)
    return out
